# revision 19
# baseline (speedup 1.0000x reference)
import numpy as np
import ml_dtypes

# ---- problem constants (hardcoded from spec) ----
B, C, H, W = 2, 128, 256, 512
P = B * H * W               # 262144 pixels
TEMPERATURE = 0.1
BASE_TEMPERATURE = 0.07
MAX_SAMPLES = 1024
MAX_VIEWS = 100
NUM_CLASSES = 8
BIG_NEG = 1e9
N = NUM_CLASSES * MAX_SAMPLES   # 8192 sampled rows
N_CORES = 8
BLK = N // N_CORES              # 1024 rows/columns per core
SCALE = np.float32(BASE_TEMPERATURE / (TEMPERATURE * TEMPERATURE))  # 7.0f exactly

_PROGRAM = {}


def _sample_indices_host(labels_flat_np):
    """Verbatim replication of reference._sample_indices on jax-CPU."""
    import jax
    import jax.numpy as jnp

    cpu = jax.devices("cpu")[0]
    with jax.default_device(cpu):
        labels_flat = jnp.asarray(labels_flat_np)
        key = jax.random.key(42)
        k1, k2 = jax.random.split(key)
        scores = jax.random.uniform(k1, (P,))
        class_mask = (
            labels_flat[None, :]
            == jnp.arange(NUM_CLASSES, dtype=labels_flat.dtype)[:, None]
        )
        masked_scores = jnp.where(class_mask, scores[None, :], -1.0)
        _, idx = jax.lax.top_k(masked_scores, MAX_SAMPLES)
        sampled_idx = idx.reshape(-1)
        row_scores = jax.random.uniform(k2, (N, MAX_SAMPLES))
        _, sel = jax.lax.top_k(row_scores, MAX_VIEWS)
        block_start = (jnp.arange(N) // MAX_SAMPLES) * MAX_SAMPLES
        pos_cols = sel + block_start[:, None]
        return np.asarray(sampled_idx), np.asarray(pos_cols)


NK = 5                  # cyclic block-columns computed per core (k = 0..4)
KC = NK * BLK           # 5120 columns of embR actually needed per core


def _chunk_tiles(c):
    """ACT tile ranges (embR col space) for chunk c under the triangle
    scheme: k0 cols [128c, 1024), k1..k3 full [1024, 4096), k4 cols
    [4096+128c, 5120).  k0 and k1..3 are contiguous -> two ranges, cut
    into <=2048-wide tiles (chunk 0's first tile split for faster start)."""
    if c == 0:
        return [(0, 1024), (1024, 2048), (2048, 4096), (4096, 5120)]
    return [(128 * c, 128 * c + 2048), (128 * c + 2048, 4096),
            (4096 + 128 * c, 5120)]


def _build_program():
    """Bass/Tile SPMD program (shared by all 8 cores).

    Triangle symmetry scheme: exp(7*G) is symmetric.  Each core computes,
    for each 128-row chunk c of its 1024-row block: the upper-triangle part
    of its own diagonal block k0 (cols >= 128c), the full k1..k3 cyclic
    blocks, and the upper-triangle part of its k4 block (whose lower part
    is covered by the partner core r+4 computing the transpose).  The
    diagonal and the doubly-counted 128x128 sub-diagonal blocks are
    corrected on the host (exact replication of the bf16-quantized dots).

    Per ACT tile: matmuls -> PSUM[128,<=2048] -> ACT exp (accum_out = row
    sums, f32) -> SBUF e-arena; DVE tensor_tensor (2x bf16) accumulates e
    into csacc (column-sum partials, partition-summed on host)."""
    if _PROGRAM:
        return _PROGRAM

    import concourse.mybir as mybir
    from concourse import bacc, tile

    f32 = mybir.dt.float32
    bf16 = mybir.dt.bfloat16
    Alu = mybir.AluOpType

    nc = bacc.Bacc("TRN2", target_bir_lowering=False)

    # embR: row-normalized embeddings, transposed [C, N], rolled so this
    # core's own 1024-column class block sits at columns 0..1023.
    embR_d = nc.dram_tensor("embR", [128, KC], bf16, kind="ExternalInput")
    cs_d = nc.dram_tensor("cs", [128, KC], bf16, kind="ExternalOutput")
    accs_d = nc.dram_tensor("accs", [128, 32], f32, kind="ExternalOutput")

    with tile.TileContext(nc) as tc:
        with (
            tc.tile_pool(name="persist", bufs=1) as persist,
            tc.tile_pool(name="psum", bufs=2, space="PSUM") as psum,
        ):
            embR = persist.tile([128, KC], bf16)
            earena = persist.tile([128, 8 * KC], bf16)   # 80KB/partition
            e4arena = persist.tile([128, 4608], bf16)    # packed k4 triangles
            scratch = persist.tile([128, 1024], bf16)    # rowsum dummy out
            csacc = persist.tile([128, KC], bf16)        # col == embR col
            accs = persist.tile([128, 32], f32)          # 4 slots per chunk

            # stream embR in; first cut unblocks the first matmul quickly
            emb_cuts = [(0, 512), (512, 1024), (1024, 2048),
                        (2048, 3072), (3072, 4096), (4096, KC)]
            for lo, hi in emb_cuts:
                nc.sync.dma_start(out=embR[:, lo:hi], in_=embR_d[:, lo:hi])

            # zero accumulators up front (DVE idle during the DMA prologue)
            nc.vector.memset(accs[:], 0.0)
            nc.vector.memset(csacc[:], 0.0)

            # Global tile order: csadds commute and accum slots are
            # per-tile, so tiles can run in any order.  With 2 PSUM
            # buffers, tile t+1's matmul fill must hide under tile t's
            # ACT — so interleave the long A/B tiles (every long fill
            # under a long ACT) and cluster the short R2 (k4-triangle)
            # tiles at the end, where short fills hide under short ACTs.
            sched = [(0, 0, 0, 1024), (0, 1, 1024, 2048)]
            for c in range(1, 8):
                # A(c), B(c-1)
                sched.append((c, 0, 128 * c, 128 * c + 2048))
                pc = c - 1
                blo = 128 * pc + 2048 if pc else 2048
                sched.append((pc, 1 + (pc == 0), blo, 4096))
            sched.append((7, 1, 128 * 7 + 2048, 4096))

            for c, slot, lo, hi in sched:
                lhsT = embR[:, c * 128:(c + 1) * 128]
                w = hi - lo
                ps = psum.tile([128, 2048], f32, tag="ps")
                for plo in range(0, w, 512):
                    pw = min(512, w - plo)
                    nc.tensor.matmul(
                        ps[:, plo:plo + pw],
                        lhsT,
                        embR[:, lo + plo: lo + plo + pw],
                        start=True, stop=True,
                    )
                e_ap = earena[:, c * KC + lo: c * KC + hi]
                nc.scalar.activation(
                    e_ap, ps[:, 0:w], mybir.ActivationFunctionType.Exp,
                    scale=float(SCALE),
                    accum_out=accs[:, c * 4 + slot: c * 4 + slot + 1],
                )
                cs_ap = csacc[:, lo:hi]
                nc.vector.tensor_tensor(
                    out=cs_ap, in0=cs_ap, in1=e_ap, op=Alu.add,
                )
                if (c, slot) == (7, 1):
                    # all k0..k3 columns final; stream out under the R2 tiles
                    nc.sync.dma_start(out=cs_d[:, 0:4096], in_=csacc[:, 0:4096])

            # k4 triangles: pair adjacent chunks into one ACT (short fills
            # hide under short ACTs).  Rows of two chunks share a tile, so
            # no ACT accum — row sums come from DVE tensor_scalar reduces
            # into the same accum slots (DVE is far from saturated here).
            e4off = [1024 * c - 64 * c * (c - 1) for c in range(9)]
            for p in range(4):
                c0, c1 = 2 * p, 2 * p + 1
                w0, w1 = 1024 - 128 * c0, 1024 - 128 * c1
                ps = psum.tile([128, 2048], f32, tag="ps")
                for i, (c, wdt) in enumerate(((c0, w0), (c1, w1))):
                    lhsT = embR[:, c * 128:(c + 1) * 128]
                    base = 0 if i == 0 else w0
                    for plo in range(0, wdt, 512):
                        pw = min(512, wdt - plo)
                        nc.tensor.matmul(
                            ps[:, base + plo: base + plo + pw],
                            lhsT,
                            embR[:, 4096 + 128 * c + plo: 4096 + 128 * c + plo + pw],
                            start=True, stop=True,
                        )
                e_ap = e4arena[:, e4off[c0]: e4off[c0] + w0 + w1]
                nc.scalar.activation(
                    e_ap, ps[:, 0:w0 + w1], mybir.ActivationFunctionType.Exp,
                    scale=float(SCALE),
                )
                for c, wdt in ((c0, w0), (c1, w1)):
                    e4_ap = e4arena[:, e4off[c]: e4off[c] + wdt]
                    slot = 3 if c == 0 else 2
                    nc.vector.tensor_scalar(
                        out=scratch[:, 0:wdt], in0=e4_ap,
                        scalar1=0.0, scalar2=0.0, op0=Alu.add, op1=Alu.add,
                        accum_out=accs[:, c * 4 + slot: c * 4 + slot + 1],
                    )
                    cs_ap = csacc[:, 4096 + 128 * c: KC]
                    nc.vector.tensor_tensor(
                        out=cs_ap, in0=cs_ap, in1=e4_ap, op=Alu.add,
                    )
            nc.sync.dma_start(out=cs_d[:, 4096:KC], in_=csacc[:, 4096:KC])

            nc.sync.dma_start(out=accs_d[:], in_=accs[:])

    nc.finalize()
    _PROGRAM["nc"] = nc
    return _PROGRAM


def _spos_host(emb_n, pos_cols):
    """s_pos = sum of exp(7*dot) over all (row, pos) pairs, excluding
    self-pairs (suppressed to exactly 0 in the reference)."""
    rows = np.repeat(np.arange(N), MAX_VIEWS)
    cols = pos_cols.ravel()
    mask = cols != rows
    rows, cols = rows[mask], cols[mask]
    total = 0.0
    for ofs in range(0, rows.size, 131072):
        r = rows[ofs:ofs + 131072]
        c = cols[ofs:ofs + 131072]
        dots = np.einsum("ij,ij->i", emb_n[r], emb_n[c], dtype=np.float64)
        total += float(np.exp(np.float64(SCALE) * dots).sum())
    return total


def _host_prep(embeddings, labels):
    sampled_idx, pos_cols = _sample_indices_host(labels.reshape(-1))
    hw = H * W
    b = sampled_idx // hw
    h = (sampled_idx % hw) // W
    w = sampled_idx % W
    emb_s = embeddings[b, :, h, w].astype(np.float32)  # [N, C]
    norm = np.sqrt(np.sum(emb_s * emb_s, axis=1, dtype=np.float32)).astype(np.float32)
    norm = np.maximum(norm, np.float32(1e-12))
    emb_n = emb_s / norm[:, None]
    embT = np.ascontiguousarray(emb_n.T).astype(ml_dtypes.bfloat16)  # [C, N]

    spos = _spos_host(emb_n, pos_cols)

    # Corrections, all with the same bf16 quantization the device matmul
    # sees: the diagonal exp(7*g_jj), plus the row sums over each row's own
    # 128-wide sub-diagonal block in its k0 block (counted twice by
    # colsum+rowsum assembly) and in its k4 block (computed by both the
    # core and its partner).
    q = embT.astype(np.float64)  # [C, N]
    s64 = np.float64(SCALE)
    diag_e = np.exp(s64 * (q * q).sum(axis=0))  # [N]
    Q = np.ascontiguousarray(q.T.reshape(64, 128, C))  # [sub, 128, C]
    G0 = np.einsum("spc,sqc->spq", Q, Q)
    ownsub0 = np.exp(s64 * G0).sum(axis=2).reshape(-1)  # [N]
    p4 = ((np.arange(64) // 8 + 4) % 8) * 8 + np.arange(64) % 8
    G4 = np.einsum("spc,sqc->spq", Q, Q[p4])
    sub4 = np.exp(s64 * G4).sum(axis=2).reshape(-1)  # [N]

    in_maps = []
    for m in range(N_CORES):
        embR = np.ascontiguousarray(np.roll(embT, -BLK * m, axis=1)[:, :KC])
        in_maps.append({"embR": embR})
    return in_maps, (spos, diag_e + ownsub0 + sub4)


def _combine(results, host_data):
    spos, corr = host_data
    rowsums, cs_k = [], []
    for res in results:
        accs = np.asarray(res["accs"], dtype=np.float64)  # [128, 32]
        rs = accs[:, 0::4] + accs[:, 1::4] + accs[:, 2::4] + accs[:, 3::4]
        rowsums.append(rs.T.reshape(-1))  # [1024], u = c*128 + p
        cs_k.append(np.asarray(res["cs"], dtype=np.float64).sum(axis=0))  # [5120]
    col_sum = np.empty(N, dtype=np.float64)
    for bblk in range(N_CORES):
        col_sum[bblk * BLK:(bblk + 1) * BLK] = (
            cs_k[bblk][0:1024]
            + cs_k[(bblk - 1) % N_CORES][1024:2048]
            + cs_k[(bblk - 2) % N_CORES][2048:3072]
            + cs_k[(bblk - 3) % N_CORES][3072:4096]
            + cs_k[(bblk + 4) % N_CORES][4096:5120]
            + rowsums[bblk]
            - corr[bblk * BLK:(bblk + 1) * BLK]
        )
    loss = -np.log(spos) + np.mean(np.log(col_sum))
    return np.float32(loss)


def kernel(embeddings: np.ndarray, labels: np.ndarray) -> np.ndarray:
    from concourse.bass_utils import run_bass_kernel_spmd

    prog = _build_program()
    in_maps, host_data = _host_prep(np.asarray(embeddings), np.asarray(labels))
    out = run_bass_kernel_spmd(prog["nc"], in_maps, list(range(N_CORES)))
    return _combine(out.results, host_data)


# revision 20
# speedup vs baseline: 1.0395x; 1.0395x over previous
import numpy as np
import ml_dtypes

# ---- problem constants (hardcoded from spec) ----
B, C, H, W = 2, 128, 256, 512
P = B * H * W               # 262144 pixels
TEMPERATURE = 0.1
BASE_TEMPERATURE = 0.07
MAX_SAMPLES = 1024
MAX_VIEWS = 100
NUM_CLASSES = 8
BIG_NEG = 1e9
N = NUM_CLASSES * MAX_SAMPLES   # 8192 sampled rows
N_CORES = 8
BLK = N // N_CORES              # 1024 rows/columns per core
SCALE = np.float32(BASE_TEMPERATURE / (TEMPERATURE * TEMPERATURE))  # 7.0f exactly

_PROGRAM = {}


def _sample_indices_host(labels_flat_np):
    """Verbatim replication of reference._sample_indices on jax-CPU."""
    import jax
    import jax.numpy as jnp

    cpu = jax.devices("cpu")[0]
    with jax.default_device(cpu):
        labels_flat = jnp.asarray(labels_flat_np)
        key = jax.random.key(42)
        k1, k2 = jax.random.split(key)
        scores = jax.random.uniform(k1, (P,))
        class_mask = (
            labels_flat[None, :]
            == jnp.arange(NUM_CLASSES, dtype=labels_flat.dtype)[:, None]
        )
        masked_scores = jnp.where(class_mask, scores[None, :], -1.0)
        _, idx = jax.lax.top_k(masked_scores, MAX_SAMPLES)
        sampled_idx = idx.reshape(-1)
        row_scores = jax.random.uniform(k2, (N, MAX_SAMPLES))
        _, sel = jax.lax.top_k(row_scores, MAX_VIEWS)
        block_start = (jnp.arange(N) // MAX_SAMPLES) * MAX_SAMPLES
        pos_cols = sel + block_start[:, None]
        return np.asarray(sampled_idx), np.asarray(pos_cols)


NK = 5                  # cyclic block-columns computed per core (k = 0..4)
KC = NK * BLK           # 5120 columns of embR actually needed per core


def _chunk_tiles(c):
    """ACT tile ranges (embR col space) for chunk c under the triangle
    scheme: k0 cols [128c, 1024), k1..k3 full [1024, 4096), k4 cols
    [4096+128c, 5120).  k0 and k1..3 are contiguous -> two ranges, cut
    into <=2048-wide tiles (chunk 0's first tile split for faster start)."""
    if c == 0:
        return [(0, 1024), (1024, 2048), (2048, 4096), (4096, 5120)]
    return [(128 * c, 128 * c + 2048), (128 * c + 2048, 4096),
            (4096 + 128 * c, 5120)]


def _build_program():
    """Bass/Tile SPMD program (shared by all 8 cores).

    Triangle symmetry scheme: exp(7*G) is symmetric.  Each core computes,
    for each 128-row chunk c of its 1024-row block: the upper-triangle part
    of its own diagonal block k0 (cols >= 128c), the full k1..k3 cyclic
    blocks, and the upper-triangle part of its k4 block (whose lower part
    is covered by the partner core r+4 computing the transpose).  The
    diagonal and the doubly-counted 128x128 sub-diagonal blocks are
    corrected on the host (exact replication of the bf16-quantized dots).

    Per ACT tile: matmuls -> PSUM[128,<=2048] -> ACT exp (accum_out = row
    sums, f32) -> SBUF e-arena; DVE tensor_tensor (2x bf16) accumulates e
    into csacc (column-sum partials, partition-summed on host)."""
    if _PROGRAM:
        return _PROGRAM

    import concourse.mybir as mybir
    from concourse import bacc, tile

    f32 = mybir.dt.float32
    bf16 = mybir.dt.bfloat16
    Alu = mybir.AluOpType

    nc = bacc.Bacc("TRN2", target_bir_lowering=False)

    # embR: row-normalized embeddings, transposed [C, N], rolled so this
    # core's own 1024-column class block sits at columns 0..1023.
    embR_d = nc.dram_tensor("embR", [128, KC], bf16, kind="ExternalInput")
    cs_d = nc.dram_tensor("cs", [128, KC], bf16, kind="ExternalOutput")
    accs_d = nc.dram_tensor("accs", [128, 32], f32, kind="ExternalOutput")

    with tile.TileContext(nc) as tc:
        with (
            tc.tile_pool(name="persist", bufs=1) as persist,
            tc.tile_pool(name="psum", bufs=2, space="PSUM") as psum,
        ):
            embR = persist.tile([128, KC], bf16)
            earena = persist.tile([128, 8 * KC], bf16)   # 80KB/partition
            e4arena = persist.tile([128, 4608], bf16)    # packed k4 triangles
            scratch = persist.tile([128, 1024], bf16)    # rowsum dummy out
            csacc = persist.tile([128, KC], bf16)        # col == embR col
            accs = persist.tile([128, 32], f32)          # 4 slots per chunk

            # stream embR in; first cut unblocks the first matmul quickly
            emb_cuts = [(0, 512), (512, 1024), (1024, 2048),
                        (2048, 3072), (3072, 4096), (4096, KC)]
            for lo, hi in emb_cuts:
                nc.sync.dma_start(out=embR[:, lo:hi], in_=embR_d[:, lo:hi])

            # zero accumulators up front (DVE idle during the DMA prologue)
            nc.vector.memset(accs[:], 0.0)
            nc.vector.memset(csacc[:], 0.0)

            # Global tile order: csadds commute and accum slots are
            # per-tile, so tiles can run in any order.  With 2 PSUM
            # buffers, tile t+1's matmul fill must hide under tile t's
            # ACT — so interleave the long A/B tiles (every long fill
            # under a long ACT) and cluster the short R2 (k4-triangle)
            # tiles at the end, where short fills hide under short ACTs.
            sched = [(0, 0, 0, 1024), (0, 1, 1024, 2048)]
            for c in range(1, 8):
                # A(c), B(c-1)
                sched.append((c, 0, 128 * c, 128 * c + 2048))
                pc = c - 1
                blo = 128 * pc + 2048 if pc else 2048
                sched.append((pc, 1 + (pc == 0), blo, 4096))
            sched.append((7, 1, 128 * 7 + 2048, 4096))

            for c, slot, lo, hi in sched:
                lhsT = embR[:, c * 128:(c + 1) * 128]
                w = hi - lo
                ps = psum.tile([128, 2048], f32, tag="ps")
                for plo in range(0, w, 512):
                    pw = min(512, w - plo)
                    nc.tensor.matmul(
                        ps[:, plo:plo + pw],
                        lhsT,
                        embR[:, lo + plo: lo + plo + pw],
                        start=True, stop=True,
                    )
                e_ap = earena[:, c * KC + lo: c * KC + hi]
                nc.scalar.activation(
                    e_ap, ps[:, 0:w], mybir.ActivationFunctionType.Exp,
                    scale=float(SCALE),
                    accum_out=accs[:, c * 4 + slot: c * 4 + slot + 1],
                )
                cs_ap = csacc[:, lo:hi]
                nc.vector.tensor_tensor(
                    out=cs_ap, in0=cs_ap, in1=e_ap, op=Alu.add,
                )
                if (c, slot) == (7, 1):
                    # all k0..k3 columns final; stream out under the R2 tiles
                    nc.sync.dma_start(out=cs_d[:, 0:4096], in_=csacc[:, 0:4096])

            # k4 triangle tiles, shortest-last: short matmul fills hide
            # under the previous (short) ACTs
            for c in range(8):
                lo, hi = 4096 + 128 * c, KC
                w = hi - lo
                lhsT = embR[:, c * 128:(c + 1) * 128]
                ps = psum.tile([128, 2048], f32, tag="ps")
                for plo in range(0, w, 512):
                    pw = min(512, w - plo)
                    nc.tensor.matmul(
                        ps[:, plo:plo + pw],
                        lhsT,
                        embR[:, lo + plo: lo + plo + pw],
                        start=True, stop=True,
                    )
                e_ap = earena[:, c * KC + lo: c * KC + hi]
                slot = 3 if c == 0 else 2
                nc.scalar.activation(
                    e_ap, ps[:, 0:w], mybir.ActivationFunctionType.Exp,
                    scale=float(SCALE),
                    accum_out=accs[:, c * 4 + slot: c * 4 + slot + 1],
                )
                cs_ap = csacc[:, lo:hi]
                nc.vector.tensor_tensor(
                    out=cs_ap, in0=cs_ap, in1=e_ap, op=Alu.add,
                )
            nc.sync.dma_start(out=cs_d[:, 4096:KC], in_=csacc[:, 4096:KC])

            nc.sync.dma_start(out=accs_d[:], in_=accs[:])

    nc.finalize()
    _PROGRAM["nc"] = nc
    return _PROGRAM


def _spos_host(emb_n, pos_cols):
    """s_pos = sum of exp(7*dot) over all (row, pos) pairs, excluding
    self-pairs (suppressed to exactly 0 in the reference)."""
    rows = np.repeat(np.arange(N), MAX_VIEWS)
    cols = pos_cols.ravel()
    mask = cols != rows
    rows, cols = rows[mask], cols[mask]
    total = 0.0
    for ofs in range(0, rows.size, 131072):
        r = rows[ofs:ofs + 131072]
        c = cols[ofs:ofs + 131072]
        dots = np.einsum("ij,ij->i", emb_n[r], emb_n[c], dtype=np.float64)
        total += float(np.exp(np.float64(SCALE) * dots).sum())
    return total


def _host_prep(embeddings, labels):
    sampled_idx, pos_cols = _sample_indices_host(labels.reshape(-1))
    hw = H * W
    b = sampled_idx // hw
    h = (sampled_idx % hw) // W
    w = sampled_idx % W
    emb_s = embeddings[b, :, h, w].astype(np.float32)  # [N, C]
    norm = np.sqrt(np.sum(emb_s * emb_s, axis=1, dtype=np.float32)).astype(np.float32)
    norm = np.maximum(norm, np.float32(1e-12))
    emb_n = emb_s / norm[:, None]
    embT = np.ascontiguousarray(emb_n.T).astype(ml_dtypes.bfloat16)  # [C, N]

    spos = _spos_host(emb_n, pos_cols)

    # Corrections, all with the same bf16 quantization the device matmul
    # sees: the diagonal exp(7*g_jj), plus the row sums over each row's own
    # 128-wide sub-diagonal block in its k0 block (counted twice by
    # colsum+rowsum assembly) and in its k4 block (computed by both the
    # core and its partner).
    q = embT.astype(np.float64)  # [C, N]
    s64 = np.float64(SCALE)
    diag_e = np.exp(s64 * (q * q).sum(axis=0))  # [N]
    Q = np.ascontiguousarray(q.T.reshape(64, 128, C))  # [sub, 128, C]
    G0 = np.einsum("spc,sqc->spq", Q, Q)
    ownsub0 = np.exp(s64 * G0).sum(axis=2).reshape(-1)  # [N]
    p4 = ((np.arange(64) // 8 + 4) % 8) * 8 + np.arange(64) % 8
    G4 = np.einsum("spc,sqc->spq", Q, Q[p4])
    sub4 = np.exp(s64 * G4).sum(axis=2).reshape(-1)  # [N]

    in_maps = []
    for m in range(N_CORES):
        embR = np.ascontiguousarray(np.roll(embT, -BLK * m, axis=1)[:, :KC])
        in_maps.append({"embR": embR})
    return in_maps, (spos, diag_e + ownsub0 + sub4)


def _combine(results, host_data):
    spos, corr = host_data
    rowsums, cs_k = [], []
    for res in results:
        accs = np.asarray(res["accs"], dtype=np.float64)  # [128, 32]
        rs = accs[:, 0::4] + accs[:, 1::4] + accs[:, 2::4] + accs[:, 3::4]
        rowsums.append(rs.T.reshape(-1))  # [1024], u = c*128 + p
        cs_k.append(np.asarray(res["cs"], dtype=np.float64).sum(axis=0))  # [5120]
    col_sum = np.empty(N, dtype=np.float64)
    for bblk in range(N_CORES):
        col_sum[bblk * BLK:(bblk + 1) * BLK] = (
            cs_k[bblk][0:1024]
            + cs_k[(bblk - 1) % N_CORES][1024:2048]
            + cs_k[(bblk - 2) % N_CORES][2048:3072]
            + cs_k[(bblk - 3) % N_CORES][3072:4096]
            + cs_k[(bblk + 4) % N_CORES][4096:5120]
            + rowsums[bblk]
            - corr[bblk * BLK:(bblk + 1) * BLK]
        )
    loss = -np.log(spos) + np.mean(np.log(col_sum))
    return np.float32(loss)


def kernel(embeddings: np.ndarray, labels: np.ndarray) -> np.ndarray:
    from concourse.bass_utils import run_bass_kernel_spmd

    prog = _build_program()
    in_maps, host_data = _host_prep(np.asarray(embeddings), np.asarray(labels))
    out = run_bass_kernel_spmd(prog["nc"], in_maps, list(range(N_CORES)))
    return _combine(out.results, host_data)


# revision 21
# speedup vs baseline: 1.1003x; 1.0585x over previous
import numpy as np
import ml_dtypes

# ---- problem constants (hardcoded from spec) ----
B, C, H, W = 2, 128, 256, 512
P = B * H * W               # 262144 pixels
TEMPERATURE = 0.1
BASE_TEMPERATURE = 0.07
MAX_SAMPLES = 1024
MAX_VIEWS = 100
NUM_CLASSES = 8
BIG_NEG = 1e9
N = NUM_CLASSES * MAX_SAMPLES   # 8192 sampled rows
N_CORES = 8
BLK = N // N_CORES              # 1024 rows/columns per core
SCALE = np.float32(BASE_TEMPERATURE / (TEMPERATURE * TEMPERATURE))  # 7.0f exactly

_PROGRAM = {}


def _sample_indices_host(labels_flat_np):
    """Verbatim replication of reference._sample_indices on jax-CPU."""
    import jax
    import jax.numpy as jnp

    cpu = jax.devices("cpu")[0]
    with jax.default_device(cpu):
        labels_flat = jnp.asarray(labels_flat_np)
        key = jax.random.key(42)
        k1, k2 = jax.random.split(key)
        scores = jax.random.uniform(k1, (P,))
        class_mask = (
            labels_flat[None, :]
            == jnp.arange(NUM_CLASSES, dtype=labels_flat.dtype)[:, None]
        )
        masked_scores = jnp.where(class_mask, scores[None, :], -1.0)
        _, idx = jax.lax.top_k(masked_scores, MAX_SAMPLES)
        sampled_idx = idx.reshape(-1)
        row_scores = jax.random.uniform(k2, (N, MAX_SAMPLES))
        _, sel = jax.lax.top_k(row_scores, MAX_VIEWS)
        block_start = (jnp.arange(N) // MAX_SAMPLES) * MAX_SAMPLES
        pos_cols = sel + block_start[:, None]
        return np.asarray(sampled_idx), np.asarray(pos_cols)


NK = 5                  # cyclic block-columns computed per core (k = 0..4)
KC = NK * BLK           # 5120 columns of embR actually needed per core


def _chunk_tiles(c):
    """ACT tile ranges (embR col space) for chunk c under the triangle
    scheme: k0 cols [128c, 1024), k1..k3 full [1024, 4096), k4 cols
    [4096+128c, 5120).  k0 and k1..3 are contiguous -> two ranges, cut
    into <=2048-wide tiles (chunk 0's first tile split for faster start)."""
    if c == 0:
        return [(0, 1024), (1024, 2048), (2048, 4096), (4096, 5120)]
    return [(128 * c, 128 * c + 2048), (128 * c + 2048, 4096),
            (4096 + 128 * c, 5120)]


def _build_program():
    """Bass/Tile SPMD program (shared by all 8 cores).

    Triangle symmetry scheme: exp(7*G) is symmetric.  Each core computes,
    for each 128-row chunk c of its 1024-row block: the upper-triangle part
    of its own diagonal block k0 (cols >= 128c), the full k1..k3 cyclic
    blocks, and the upper-triangle part of its k4 block (whose lower part
    is covered by the partner core r+4 computing the transpose).  The
    diagonal and the doubly-counted 128x128 sub-diagonal blocks are
    corrected on the host (exact replication of the bf16-quantized dots).

    Per ACT tile: matmuls -> PSUM[128,<=2048] -> ACT exp (accum_out = row
    sums, f32) -> SBUF e-arena; DVE tensor_tensor (2x bf16) accumulates e
    into csacc (column-sum partials, partition-summed on host)."""
    if _PROGRAM:
        return _PROGRAM

    import concourse.mybir as mybir
    from concourse import bacc, tile

    f32 = mybir.dt.float32
    bf16 = mybir.dt.bfloat16
    Alu = mybir.AluOpType

    nc = bacc.Bacc("TRN2", target_bir_lowering=False)

    # embR: row-normalized embeddings, transposed [C, N], rolled so this
    # core's own 1024-column class block sits at columns 0..1023.
    embR_d = nc.dram_tensor("embR", [128, KC], bf16, kind="ExternalInput")
    cs_d = nc.dram_tensor("cs", [128, KC], bf16, kind="ExternalOutput")
    accs_d = nc.dram_tensor("accs", [128, 32], f32, kind="ExternalOutput")

    with tile.TileContext(nc) as tc:
        with (
            tc.tile_pool(name="persist", bufs=1) as persist,
            tc.tile_pool(name="psum", bufs=2, space="PSUM") as psum,
        ):
            embR = persist.tile([128, KC], bf16)
            earena = persist.tile([128, 8 * KC], bf16)   # 80KB/partition
            csacc = persist.tile([128, KC], bf16)        # col == embR col
            accs = persist.tile([128, 32], f32)          # 4 slots per chunk

            # stream embR in; first cut unblocks the first matmul quickly
            emb_cuts = [(0, 512), (512, 1024), (1024, 2048),
                        (2048, 3072), (3072, 4096), (4096, KC)]
            for lo, hi in emb_cuts:
                nc.sync.dma_start(out=embR[:, lo:hi], in_=embR_d[:, lo:hi])

            # zero accumulators up front (DVE idle during the DMA prologue)
            nc.vector.memset(accs[:], 0.0)
            nc.vector.memset(csacc[:], 0.0)

            # Global tile order: csadds commute and accum slots are
            # per-tile, so tiles can run in any order.  With 2 PSUM
            # buffers, tile t+1's matmul fill must hide under tile t's
            # ACT — so interleave the long A/B tiles (every long fill
            # under a long ACT) and cluster the short R2 (k4-triangle)
            # tiles at the end, where short fills hide under short ACTs.
            sched = [(0, 0, 0, 1024), (0, 1, 1024, 2048)]
            for c in range(1, 8):
                # A(c), B(c-1)
                sched.append((c, 0, 128 * c, 128 * c + 2048))
                pc = c - 1
                blo = 128 * pc + 2048 if pc else 2048
                sched.append((pc, 1 + (pc == 0), blo, 4096))
            sched.append((7, 1, 128 * 7 + 2048, 4096))

            for c, slot, lo, hi in sched:
                lhsT = embR[:, c * 128:(c + 1) * 128]
                w = hi - lo
                ps = psum.tile([128, 2048], f32, tag="ps")
                for plo in range(0, w, 512):
                    pw = min(512, w - plo)
                    nc.tensor.matmul(
                        ps[:, plo:plo + pw],
                        lhsT,
                        embR[:, lo + plo: lo + plo + pw],
                        start=True, stop=True,
                    )
                e_ap = earena[:, c * KC + lo: c * KC + hi]
                nc.scalar.activation(
                    e_ap, ps[:, 0:w], mybir.ActivationFunctionType.Exp,
                    scale=float(SCALE),
                    accum_out=accs[:, c * 4 + slot: c * 4 + slot + 1],
                )
                cs_ap = csacc[:, lo:hi]
                nc.vector.tensor_tensor(
                    out=cs_ap, in0=cs_ap, in1=e_ap, op=Alu.add,
                )
                if (c, slot) == (7, 1):
                    # all k0..k3 columns final; stream out under the R2 tiles
                    nc.sync.dma_start(out=cs_d[:, 0:4096], in_=csacc[:, 0:4096])

            # k4 triangle tiles, shortest-last: short matmul fills hide
            # under the previous (short) ACTs
            for c in range(8):
                lo, hi = 4096 + 128 * c, KC
                w = hi - lo
                lhsT = embR[:, c * 128:(c + 1) * 128]
                ps = psum.tile([128, 2048], f32, tag="ps")
                for plo in range(0, w, 512):
                    pw = min(512, w - plo)
                    nc.tensor.matmul(
                        ps[:, plo:plo + pw],
                        lhsT,
                        embR[:, lo + plo: lo + plo + pw],
                        start=True, stop=True,
                    )
                e_ap = earena[:, c * KC + lo: c * KC + hi]
                slot = 3 if c == 0 else 2
                nc.scalar.activation(
                    e_ap, ps[:, 0:w], mybir.ActivationFunctionType.Exp,
                    scale=float(SCALE),
                    accum_out=accs[:, c * 4 + slot: c * 4 + slot + 1],
                )
                cs_ap = csacc[:, lo:hi]
                nc.vector.tensor_tensor(
                    out=cs_ap, in0=cs_ap, in1=e_ap, op=Alu.add,
                )
            nc.sync.dma_start(out=cs_d[:, 4096:KC], in_=csacc[:, 4096:KC])

            nc.sync.dma_start(out=accs_d[:], in_=accs[:])

    nc.finalize()
    _PROGRAM["nc"] = nc
    return _PROGRAM


def _spos_host(emb_n, pos_cols):
    """s_pos = sum of exp(7*dot) over all (row, pos) pairs, excluding
    self-pairs (suppressed to exactly 0 in the reference)."""
    rows = np.repeat(np.arange(N), MAX_VIEWS)
    cols = pos_cols.ravel()
    mask = cols != rows
    rows, cols = rows[mask], cols[mask]
    total = 0.0
    for ofs in range(0, rows.size, 131072):
        r = rows[ofs:ofs + 131072]
        c = cols[ofs:ofs + 131072]
        dots = np.einsum("ij,ij->i", emb_n[r], emb_n[c], dtype=np.float64)
        total += float(np.exp(np.float64(SCALE) * dots).sum())
    return total


def _host_prep(embeddings, labels):
    sampled_idx, pos_cols = _sample_indices_host(labels.reshape(-1))
    hw = H * W
    b = sampled_idx // hw
    h = (sampled_idx % hw) // W
    w = sampled_idx % W
    emb_s = embeddings[b, :, h, w].astype(np.float32)  # [N, C]
    norm = np.sqrt(np.sum(emb_s * emb_s, axis=1, dtype=np.float32)).astype(np.float32)
    norm = np.maximum(norm, np.float32(1e-12))
    emb_n = emb_s / norm[:, None]
    embT = np.ascontiguousarray(emb_n.T).astype(ml_dtypes.bfloat16)  # [C, N]

    spos = _spos_host(emb_n, pos_cols)

    # Corrections, all with the same bf16 quantization the device matmul
    # sees: the diagonal exp(7*g_jj), plus the row sums over each row's own
    # 128-wide sub-diagonal block in its k0 block (counted twice by
    # colsum+rowsum assembly) and in its k4 block (computed by both the
    # core and its partner).
    q = embT.astype(np.float64)  # [C, N]
    s64 = np.float64(SCALE)
    diag_e = np.exp(s64 * (q * q).sum(axis=0))  # [N]
    Q = np.ascontiguousarray(q.T.reshape(64, 128, C))  # [sub, 128, C]
    G0 = np.einsum("spc,sqc->spq", Q, Q)
    ownsub0 = np.exp(s64 * G0).sum(axis=2).reshape(-1)  # [N]
    p4 = ((np.arange(64) // 8 + 4) % 8) * 8 + np.arange(64) % 8
    G4 = np.einsum("spc,sqc->spq", Q, Q[p4])
    sub4 = np.exp(s64 * G4).sum(axis=2).reshape(-1)  # [N]

    in_maps = []
    for m in range(N_CORES):
        embR = np.ascontiguousarray(np.roll(embT, -BLK * m, axis=1)[:, :KC])
        in_maps.append({"embR": embR})
    return in_maps, (spos, diag_e + ownsub0 + sub4)


def _combine(results, host_data):
    spos, corr = host_data
    rowsums, cs_k = [], []
    for res in results:
        accs = np.asarray(res["accs"], dtype=np.float64)  # [128, 32]
        rs = accs[:, 0::4] + accs[:, 1::4] + accs[:, 2::4] + accs[:, 3::4]
        rowsums.append(rs.T.reshape(-1))  # [1024], u = c*128 + p
        cs_k.append(np.asarray(res["cs"], dtype=np.float64).sum(axis=0))  # [5120]
    col_sum = np.empty(N, dtype=np.float64)
    for bblk in range(N_CORES):
        col_sum[bblk * BLK:(bblk + 1) * BLK] = (
            cs_k[bblk][0:1024]
            + cs_k[(bblk - 1) % N_CORES][1024:2048]
            + cs_k[(bblk - 2) % N_CORES][2048:3072]
            + cs_k[(bblk - 3) % N_CORES][3072:4096]
            + cs_k[(bblk + 4) % N_CORES][4096:5120]
            + rowsums[bblk]
            - corr[bblk * BLK:(bblk + 1) * BLK]
        )
    loss = -np.log(spos) + np.mean(np.log(col_sum))
    return np.float32(loss)


def kernel(embeddings: np.ndarray, labels: np.ndarray) -> np.ndarray:
    from concourse.bass_utils import run_bass_kernel_spmd

    prog = _build_program()
    in_maps, host_data = _host_prep(np.asarray(embeddings), np.asarray(labels))
    out = run_bass_kernel_spmd(prog["nc"], in_maps, list(range(N_CORES)))
    return _combine(out.results, host_data)


# revision 23
# speedup vs baseline: 1.1112x; 1.0099x over previous
import numpy as np
import ml_dtypes

# ---- problem constants (hardcoded from spec) ----
B, C, H, W = 2, 128, 256, 512
P = B * H * W               # 262144 pixels
TEMPERATURE = 0.1
BASE_TEMPERATURE = 0.07
MAX_SAMPLES = 1024
MAX_VIEWS = 100
NUM_CLASSES = 8
BIG_NEG = 1e9
N = NUM_CLASSES * MAX_SAMPLES   # 8192 sampled rows
N_CORES = 8
BLK = N // N_CORES              # 1024 rows/columns per core
SCALE = np.float32(BASE_TEMPERATURE / (TEMPERATURE * TEMPERATURE))  # 7.0f exactly

_PROGRAM = {}


def _sample_indices_host(labels_flat_np):
    """Verbatim replication of reference._sample_indices on jax-CPU."""
    import jax
    import jax.numpy as jnp

    cpu = jax.devices("cpu")[0]
    with jax.default_device(cpu):
        labels_flat = jnp.asarray(labels_flat_np)
        key = jax.random.key(42)
        k1, k2 = jax.random.split(key)
        scores = jax.random.uniform(k1, (P,))
        class_mask = (
            labels_flat[None, :]
            == jnp.arange(NUM_CLASSES, dtype=labels_flat.dtype)[:, None]
        )
        masked_scores = jnp.where(class_mask, scores[None, :], -1.0)
        _, idx = jax.lax.top_k(masked_scores, MAX_SAMPLES)
        sampled_idx = idx.reshape(-1)
        row_scores = jax.random.uniform(k2, (N, MAX_SAMPLES))
        _, sel = jax.lax.top_k(row_scores, MAX_VIEWS)
        block_start = (jnp.arange(N) // MAX_SAMPLES) * MAX_SAMPLES
        pos_cols = sel + block_start[:, None]
        return np.asarray(sampled_idx), np.asarray(pos_cols)


NK = 5                  # cyclic block-columns computed per core (k = 0..4)
KC = NK * BLK           # 5120 columns of embR actually needed per core


def _chunk_tiles(c):
    """ACT tile ranges (embR col space) for chunk c under the triangle
    scheme: k0 cols [128c, 1024), k1..k3 full [1024, 4096), k4 cols
    [4096+128c, 5120).  k0 and k1..3 are contiguous -> two ranges, cut
    into <=2048-wide tiles (chunk 0's first tile split for faster start)."""
    if c == 0:
        return [(0, 1024), (1024, 2048), (2048, 4096), (4096, 5120)]
    return [(128 * c, 128 * c + 2048), (128 * c + 2048, 4096),
            (4096 + 128 * c, 5120)]


def _build_program():
    """Bass/Tile SPMD program (shared by all 8 cores).

    Triangle symmetry scheme: exp(7*G) is symmetric.  Each core computes,
    for each 128-row chunk c of its 1024-row block: the upper-triangle part
    of its own diagonal block k0 (cols >= 128c), the full k1..k3 cyclic
    blocks, and the upper-triangle part of its k4 block (whose lower part
    is covered by the partner core r+4 computing the transpose).  The
    diagonal and the doubly-counted 128x128 sub-diagonal blocks are
    corrected on the host (exact replication of the bf16-quantized dots).

    Per ACT tile: matmuls -> PSUM[128,<=2048] -> ACT exp (accum_out = row
    sums, f32) -> SBUF e-arena; DVE tensor_tensor (2x bf16) accumulates e
    into csacc (column-sum partials, partition-summed on host)."""
    if _PROGRAM:
        return _PROGRAM

    import concourse.mybir as mybir
    from concourse import bacc, tile

    f32 = mybir.dt.float32
    bf16 = mybir.dt.bfloat16
    Alu = mybir.AluOpType

    nc = bacc.Bacc("TRN2", target_bir_lowering=False)

    # embR: row-normalized embeddings, transposed [C, N], rolled so this
    # core's own 1024-column class block sits at columns 0..1023.
    embR_d = nc.dram_tensor("embR", [128, KC], bf16, kind="ExternalInput")
    cs_d = nc.dram_tensor("cs", [128, KC], bf16, kind="ExternalOutput")
    accs_d = nc.dram_tensor("accs", [128, 32], f32, kind="ExternalOutput")

    with tile.TileContext(nc) as tc:
        with (
            tc.tile_pool(name="persist", bufs=1) as persist,
            tc.tile_pool(name="psum", bufs=2, space="PSUM") as psum,
        ):
            embR = persist.tile([128, KC], bf16)
            earena = persist.tile([128, 8 * KC], bf16)   # 80KB/partition
            csacc = persist.tile([128, KC], bf16)        # col == embR col
            accs = persist.tile([128, 32], f32)          # 4 slots per chunk

            # stream embR in; first cut unblocks the first matmul quickly
            emb_cuts = [(0, 512), (512, 1024), (1024, 2048),
                        (2048, 3072), (3072, 4096), (4096, KC)]
            for lo, hi in emb_cuts:
                nc.sync.dma_start(out=embR[:, lo:hi], in_=embR_d[:, lo:hi])

            # zero accumulators up front (DVE idle during the DMA prologue)
            nc.vector.memset(accs[:], 0.0)
            nc.vector.memset(csacc[:], 0.0)

            # Global tile order: csadds commute and accum slots are
            # per-tile, so tiles can run in any order.  With 2 PSUM
            # buffers, tile t+1's matmul fill must hide under tile t's
            # ACT — so interleave the long A/B tiles (every long fill
            # under a long ACT) and cluster the short R2 (k4-triangle)
            # tiles at the end, where short fills hide under short ACTs.
            sched = [(0, 0, 0, 1024), (0, 1, 1024, 2048)]
            for c in range(1, 8):
                # A(c), B(c-1)
                sched.append((c, 0, 128 * c, 128 * c + 2048))
                pc = c - 1
                blo = 128 * pc + 2048 if pc else 2048
                sched.append((pc, 1 + (pc == 0), blo, 4096))
            sched.append((7, 1, 128 * 7 + 2048, 4096))

            for c, slot, lo, hi in sched:
                lhsT = embR[:, c * 128:(c + 1) * 128]
                w = hi - lo
                ps = psum.tile([128, 2048], f32, tag="ps")
                for plo in range(0, w, 512):
                    pw = min(512, w - plo)
                    nc.tensor.matmul(
                        ps[:, plo:plo + pw],
                        lhsT,
                        embR[:, lo + plo: lo + plo + pw],
                        start=True, stop=True,
                    )
                e_ap = earena[:, c * KC + lo: c * KC + hi]
                nc.scalar.activation(
                    e_ap, ps[:, 0:w], mybir.ActivationFunctionType.Exp,
                    scale=float(SCALE),
                    accum_out=accs[:, c * 4 + slot: c * 4 + slot + 1],
                )
                cs_ap = csacc[:, lo:hi]
                nc.vector.tensor_tensor(
                    out=cs_ap, in0=cs_ap, in1=e_ap, op=Alu.add,
                )
                if (c, slot) == (7, 1):
                    # all k0..k3 columns final; stream out under the R2 tiles
                    nc.sync.dma_start(out=cs_d[:, 0:4096], in_=csacc[:, 0:4096])

            # k4 strict-triangle tiles (sub-diagonal 128-blocks are added
            # back on the host), shortest-last: short matmul fills hide
            # under the previous (short) ACTs
            for c in range(7):
                lo, hi = 4096 + 128 * (c + 1), KC
                w = hi - lo
                lhsT = embR[:, c * 128:(c + 1) * 128]
                ps = psum.tile([128, 2048], f32, tag="ps")
                for plo in range(0, w, 512):
                    pw = min(512, w - plo)
                    nc.tensor.matmul(
                        ps[:, plo:plo + pw],
                        lhsT,
                        embR[:, lo + plo: lo + plo + pw],
                        start=True, stop=True,
                    )
                e_ap = earena[:, c * KC + lo: c * KC + hi]
                slot = 3 if c == 0 else 2
                nc.scalar.activation(
                    e_ap, ps[:, 0:w], mybir.ActivationFunctionType.Exp,
                    scale=float(SCALE),
                    accum_out=accs[:, c * 4 + slot: c * 4 + slot + 1],
                )
                cs_ap = csacc[:, lo:hi]
                nc.vector.tensor_tensor(
                    out=cs_ap, in0=cs_ap, in1=e_ap, op=Alu.add,
                )
            nc.sync.dma_start(out=cs_d[:, 4096:KC], in_=csacc[:, 4096:KC])

            nc.sync.dma_start(out=accs_d[:], in_=accs[:])

    nc.finalize()
    _PROGRAM["nc"] = nc
    return _PROGRAM


def _spos_host(emb_n, pos_cols):
    """s_pos = sum of exp(7*dot) over all (row, pos) pairs, excluding
    self-pairs (suppressed to exactly 0 in the reference)."""
    rows = np.repeat(np.arange(N), MAX_VIEWS)
    cols = pos_cols.ravel()
    mask = cols != rows
    rows, cols = rows[mask], cols[mask]
    total = 0.0
    for ofs in range(0, rows.size, 131072):
        r = rows[ofs:ofs + 131072]
        c = cols[ofs:ofs + 131072]
        dots = np.einsum("ij,ij->i", emb_n[r], emb_n[c], dtype=np.float64)
        total += float(np.exp(np.float64(SCALE) * dots).sum())
    return total


def _host_prep(embeddings, labels):
    sampled_idx, pos_cols = _sample_indices_host(labels.reshape(-1))
    hw = H * W
    b = sampled_idx // hw
    h = (sampled_idx % hw) // W
    w = sampled_idx % W
    emb_s = embeddings[b, :, h, w].astype(np.float32)  # [N, C]
    norm = np.sqrt(np.sum(emb_s * emb_s, axis=1, dtype=np.float32)).astype(np.float32)
    norm = np.maximum(norm, np.float32(1e-12))
    emb_n = emb_s / norm[:, None]
    embT = np.ascontiguousarray(emb_n.T).astype(ml_dtypes.bfloat16)  # [C, N]

    spos = _spos_host(emb_n, pos_cols)

    # Corrections, all with the same bf16 quantization the device matmul
    # sees: the diagonal exp(7*g_jj), plus the row sums over each row's own
    # 128-wide sub-diagonal block in its k0 block (counted twice by
    # colsum+rowsum assembly) and in its k4 block (computed by both the
    # core and its partner).
    q = embT.astype(np.float64)  # [C, N]
    s64 = np.float64(SCALE)
    diag_e = np.exp(s64 * (q * q).sum(axis=0))  # [N]
    Q = np.ascontiguousarray(q.T.reshape(64, 128, C))  # [sub, 128, C]
    G0 = np.einsum("spc,sqc->spq", Q, Q)
    ownsub0 = np.exp(s64 * G0).sum(axis=2).reshape(-1)  # [N]
    p4 = ((np.arange(64) // 8 + 4) % 8) * 8 + np.arange(64) % 8
    G4 = np.einsum("spc,sqc->spq", Q, Q[p4])
    sub4 = np.exp(s64 * G4).sum(axis=2).reshape(-1)  # [N]

    in_maps = []
    for m in range(N_CORES):
        embR = np.ascontiguousarray(np.roll(embT, -BLK * m, axis=1)[:, :KC])
        in_maps.append({"embR": embR})
    # k4 sub-diagonal blocks are computed by neither partner core -> ADD
    # sub4 back; k0 sub-diagonals are double-counted -> subtract ownsub0.
    return in_maps, (spos, diag_e + ownsub0 - sub4)


def _combine(results, host_data):
    spos, corr = host_data
    rowsums, cs_k = [], []
    for res in results:
        accs = np.asarray(res["accs"], dtype=np.float64)  # [128, 32]
        rs = accs[:, 0::4] + accs[:, 1::4] + accs[:, 2::4] + accs[:, 3::4]
        rowsums.append(rs.T.reshape(-1))  # [1024], u = c*128 + p
        cs_k.append(np.asarray(res["cs"], dtype=np.float64).sum(axis=0))  # [5120]
    col_sum = np.empty(N, dtype=np.float64)
    for bblk in range(N_CORES):
        col_sum[bblk * BLK:(bblk + 1) * BLK] = (
            cs_k[bblk][0:1024]
            + cs_k[(bblk - 1) % N_CORES][1024:2048]
            + cs_k[(bblk - 2) % N_CORES][2048:3072]
            + cs_k[(bblk - 3) % N_CORES][3072:4096]
            + cs_k[(bblk + 4) % N_CORES][4096:5120]
            + rowsums[bblk]
            - corr[bblk * BLK:(bblk + 1) * BLK]
        )
    loss = -np.log(spos) + np.mean(np.log(col_sum))
    return np.float32(loss)


def kernel(embeddings: np.ndarray, labels: np.ndarray) -> np.ndarray:
    from concourse.bass_utils import run_bass_kernel_spmd

    prog = _build_program()
    in_maps, host_data = _host_prep(np.asarray(embeddings), np.asarray(labels))
    out = run_bass_kernel_spmd(prog["nc"], in_maps, list(range(N_CORES)))
    return _combine(out.results, host_data)
